# revision 22
# baseline (speedup 1.0000x reference)
"""Bidirectional-LSTM basecaller on 8 Trainium2 NeuronCores (Bass/Tile).

Sharding: cores 0-3 run the FORWARD LSTM for 8 sequences each; cores 4-7 run
the BACKWARD direction of the same sequences on host-reversed signals (conv
kernels flipped host-side, which commutes with SAME conv). Variable lengths
are handled uniformly: each backward lane's signal is rotated so real data
starts at step 1; step 0 consumes one garbage conv window whose state
pollution is killed by adding -40 to the i-gate input projection at step 0
(c0 ~ 0, h0 ~ 0). One SPMD program, per-core input data only.

On-core: fused streaming pipeline over 512-col t-blocks:
  conv2 (K=3, 6 accum matmuls) -> conv3 + conv1a (host rank-1) -> gx =
  enc @ Wx + b (f16 to DRAM); LSTM chunks (CT steps) + chunk-wise dense
  decode start as soon as their gx lands, so all bulk matmul work overlaps
  the recurrence's idle engine time. The recurrent step: PE pre-fills PSUM
  with gx[t] via identity matmul then accumulates 16 [128|72 x 128] x [.,8]
  f16 matmuls; gates use a single sigmoid (j-columns pre-scaled x2,
  tanh(x) = 2*sigmoid(2x) - 1); tanh(c) likewise via sigmoid(2c).
  h lives in a rolling (CT+1)-step window.

Host: pack shards, run SPMD via run_bass_kernel_spmd, assemble logits
(backward time-remap + bias + length masking).
"""
import os
import sys
import numpy as np

for _p in ("/opt/trn_rl_repo", "/root/.axon_site/_ro/trn_rl_repo"):
    if os.path.isdir(_p) and _p not in sys.path:
        sys.path.insert(0, _p)

import ml_dtypes  # noqa: F401,E402

B, T, H, C = 32, 2048, 200, 256
GH = 256                # padded per-gate width
G4 = 4 * GH             # 1024
LANES = 8
NB = 512                # matmul free-dim block
NP16 = np.float16       # 16-bit dtype for weights/activations

_CACHE = {}


# ---------------------------------------------------------------- bass build
def _build(TS, CT):
    """SPMD bass program: fused streaming pipeline over 512-col t-blocks."""
    import concourse.bass as bass  # noqa: F401
    import concourse.tile as tile
    from concourse import bacc, mybir

    f32 = mybir.dt.float32
    f16 = mybir.dt.float16

    SIGP = TS + 2
    KT = [128, H - 128]

    nc = bacc.Bacc("TRN2", target_bir_lowering=False, debug=False, num_devices=8)

    c1t_d = nc.dram_tensor("c1t", [2, 128, LANES * SIGP], f16, kind="ExternalInput")
    enc0_d = nc.dram_tensor("enc0", [2, 128, LANES * TS], f16, kind="ExternalInput")
    k2_d = nc.dram_tensor("k2", [2, 3, 2, 128, 128], f16, kind="ExternalInput")
    k3_d = nc.dram_tensor("k3", [2, 2, 128, 128], f16, kind="ExternalInput")
    wx_d = nc.dram_tensor("wx", [2, 128, G4], f16, kind="ExternalInput")
    wh_d = nc.dram_tensor("wh", [2, 128, G4], f16, kind="ExternalInput")
    gb_d = nc.dram_tensor("gb", [128, 8], f32, kind="ExternalInput")
    wd_d = nc.dram_tensor("wd", [2, 128, 5], f16, kind="ExternalInput")
    patch_d = nc.dram_tensor("patch", [128, 2, 8], f16, kind="ExternalInput")
    id_d = nc.dram_tensor("id128", [128, 128], f16, kind="ExternalInput")

    gx_d = nc.dram_tensor("gx", [8, 128, LANES, TS], f16, kind="Internal")
    part_d = nc.dram_tensor("part", [5, TS, LANES], f32, kind="ExternalOutput")

    tblks = [(i * NB, min(NB, TS - i * NB)) for i in range((TS + NB - 1) // NB)]
    chks = [(i * CT, min(CT, TS - i * CT)) for i in range((TS + CT - 1) // CT)]

    c1v = c1t_d.ap().rearrange("a p (l t) -> a p l t", l=LANES)
    e0v = enc0_d.ap().rearrange("a p (l t) -> a p l t", l=LANES)

    with tile.TileContext(nc) as tc:
        with (
            tc.tile_pool(name="seq", bufs=1) as seqp,
            tc.tile_pool(name="wts", bufs=1) as wtsp,
            tc.tile_pool(name="c1p", bufs=2) as c1p,
            tc.tile_pool(name="c2p", bufs=2) as c2p,
            tc.tile_pool(name="encp", bufs=2) as encp,
            tc.tile_pool(name="stage", bufs=3) as stagep,
            tc.tile_pool(name="gxbp", bufs=2) as gxbp,
            tc.tile_pool(name="hwp", bufs=2) as hwp,
            tc.tile_pool(name="gat", bufs=3) as gatp,
            tc.tile_pool(name="psA", bufs=4, space="PSUM") as psA,
            tc.tile_pool(name="zp", bufs=2, space="PSUM") as zp,
            tc.tile_pool(name="psF", bufs=2, space="PSUM") as psF,
        ):
            # ---------------- weights to SBUF
            k2_s = wtsp.tile([128, 2, 3, 2, 128], f16)
            nc.sync.dma_start(k2_s[:], k2_d.ap().rearrange("a k c p m -> p a k c m"))
            k3_s = wtsp.tile([128, 2, 2, 128], f16)
            nc.sync.dma_start(k3_s[:], k3_d.ap().rearrange("a c p m -> p a c m"))
            wx_s = wtsp.tile([128, 2, G4], f16)
            nc.sync.dma_start(wx_s[:], wx_d.ap().rearrange("k p m -> p k m"))
            gb_s = wtsp.tile([128, 8], f32)
            nc.sync.dma_start(gb_s[:], gb_d.ap())
            wh_s = seqp.tile([128, 2, G4], f16)
            nc.sync.dma_start(wh_s[:], wh_d.ap().rearrange("k p m -> p k m"))
            wd_s = seqp.tile([128, 2, 5], f16)
            nc.sync.dma_start(wd_s[:], wd_d.ap().rearrange("k p m -> p k m"))
            patch_s = seqp.tile([128, 2, 8], f16)
            nc.sync.dma_start(patch_s[:], patch_d.ap())
            id_s = seqp.tile([128, 128], f16)
            nc.sync.dma_start(id_s[:], id_d.ap())

            c_sb = seqp.tile([128, 2, LANES], f32, tag="c")
            nc.vector.memset(c_sb[:], 0.0)
            cf = c_sb[:].rearrange("p a l -> p (a l)")

            Mu = mybir.AluOpType.mult
            Ad = mybir.AluOpType.add
            Su = mybir.AluOpType.subtract
            Relu = mybir.ActivationFunctionType.Relu
            Sig = mybir.ActivationFunctionType.Sigmoid
            Ident = mybir.ActivationFunctionType.Identity

            state = {"win": None}

            def emit_chunk(c):
                o_c, n_c = chks[c]
                gxb = gxbp.tile([128, 8, LANES, CT], f16, tag="gxb")
                for mi in range(8):
                    nc.sync.dma_start(gxb[:, mi, :, :n_c],
                                      gx_d.ap()[mi, :, :, o_c:o_c + n_c])
                if c == 0:
                    nc.vector.tensor_add(gxb[:, 2:4, :, 0],
                                         gxb[:, 2:4, :, 0], patch_s[:])
                win = hwp.tile([128, CT + 1, 2, LANES], f16, tag="win")
                if c == 0:
                    nc.vector.memset(win[:, 0, :, :], 0.0)
                else:
                    nc.vector.tensor_copy(
                        win[:, 0, :, :].rearrange("p a l -> p (a l)"),
                        state["win"][:, CT, :, :].rearrange("p a l -> p (a l)"))
                for lt in range(n_c):
                    z = zp.tile([128, 64], f32, tag="z")
                    nc.tensor.matmul(
                        z[:].rearrange("p (m l) -> p m l", m=8),
                        id_s[:], gxb[:, :, :, lt], start=True, stop=False)
                    for m in range(8):
                        for k in range(2):
                            kn = KT[k]
                            nc.tensor.matmul(
                                z[:, m * 8:(m + 1) * 8],
                                wh_s[0:kn, k, m * 128:(m + 1) * 128],
                                win[0:kn, lt, k, :],
                                start=False, stop=(m == 7 and k == 1),
                                skip_group_check=True)
                    # gates: one sigmoid (j cols pre-scaled x2: tanh=2sig-1)
                    gt = gatp.tile([128, 64], f32, tag="gt")
                    nc.scalar.activation(gt[:], z[:], Sig)
                    jt = gatp.tile([128, 16], f32, tag="jt")
                    nc.vector.tensor_scalar(jt[:], gt[:, 0:16], 2.0, -1.0, Mu, Ad)
                    tmp = gatp.tile([128, 16], f32, tag="tmp")
                    nc.vector.tensor_mul(tmp[:], jt[:], gt[:, 16:32])
                    nc.vector.tensor_mul(cf, cf, gt[:, 32:48])
                    nc.vector.tensor_add(cf, cf, tmp[:])
                    tc_t = gatp.tile([128, 16], f32, tag="tanc")
                    nc.scalar.activation(tc_t[:], cf, Sig, scale=2.0)
                    q = gatp.tile([128, 16], f32, tag="q")
                    nc.vector.tensor_mul(q[:], tc_t[:], gt[:, 48:64])
                    nc.vector.scalar_tensor_tensor(
                        win[:, lt + 1, :, :].rearrange("p a l -> p (a l)"),
                        q[:], 2.0, gt[:, 48:64], Mu, Su)
                # chunk-wise dense decode
                for fo in range(0, n_c, 64):
                    fn = min(64, n_c - fo)
                    psf = psF.tile([5, NB], f32, tag="psf")
                    for k in range(2):
                        kn = KT[k]
                        nc.tensor.matmul(
                            psf[:, :fn * 8].rearrange("p (t l) -> p t l", l=8),
                            wd_s[0:kn, k, :],
                            win[0:kn, 1 + fo:1 + fo + fn, k, :],
                            start=(k == 0), stop=(k == 1))
                    stf = stagep.tile([5, NB], f32, tag="stf")
                    nc.vector.tensor_copy(stf[:, :fn * 8], psf[:, :fn * 8])
                    nc.sync.dma_start(
                        part_d.ap()[:, o_c + fo:o_c + fo + fn, :],
                        stf[:, :fn * 8].rearrange("p (t l) -> p t l", l=8))
                state["win"] = win

            chunks_done = 0
            for tbi, (o, n) in enumerate(tblks):
                nh = min(n + 2, SIGP - o)
                c1b = c1p.tile([128, 2, LANES, NB + 2], f16, tag="c1")
                for ci in range(2):
                    nc.sync.dma_start(c1b[:, ci, :, :nh], c1v[ci, :, :, o:o + nh])
                c2b = c2p.tile([128, 2, LANES, NB], f16, tag="c2")
                for co in range(2):
                    for ln in range(LANES):
                        ps = psA.tile([128, NB], f32, tag="psA")
                        first = True
                        for k in range(3):
                            for ci in range(2):
                                nc.tensor.matmul(
                                    ps[:, :n], k2_s[:, co, k, ci, :],
                                    c1b[:, ci, ln, k:k + n],
                                    start=first, stop=(k == 2 and ci == 1))
                                first = False
                        nc.scalar.activation(c2b[:, co, ln, :n], ps[:, :n], Relu)
                encb = encp.tile([128, 2, LANES, NB], f16, tag="enc")
                for co in range(2):
                    nc.sync.dma_start(encb[:, co, :, :n], e0v[co, :, :, o:o + n])
                for co in range(2):
                    for ln in range(LANES):
                        ps3 = psA.tile([128, NB], f32, tag="psA")
                        for ci in range(2):
                            nc.tensor.matmul(ps3[:, :n], k3_s[:, co, ci, :],
                                             c2b[:, ci, ln, :n],
                                             start=(ci == 0), stop=(ci == 1))
                        t3 = stagep.tile([128, NB], f16, tag="st3")
                        nc.scalar.activation(t3[:, :n], ps3[:, :n], Relu)
                        nc.vector.tensor_add(encb[:, co, ln, :n],
                                             encb[:, co, ln, :n], t3[:, :n])
                for m in range(8):
                    for ln in range(LANES):
                        ps = psA.tile([128, NB], f32, tag="psA")
                        for ci in range(2):
                            nc.tensor.matmul(ps[:, :n],
                                             wx_s[:, ci, m * 128:(m + 1) * 128],
                                             encb[:, ci, ln, :n],
                                             start=(ci == 0), stop=(ci == 1))
                        st = stagep.tile([128, NB], f16, tag="stgx")
                        nc.scalar.activation(st[:, :n], ps[:, :n], Ident,
                                             bias=gb_s[:, m:m + 1])
                        nc.sync.dma_start(gx_d.ap()[m, :, ln, o:o + n], st[:, :n])
                while (chunks_done < len(chks)
                       and chks[chunks_done][0] + chks[chunks_done][1] <= o + n):
                    emit_chunk(chunks_done)
                    chunks_done += 1
            while chunks_done < len(chks):
                emit_chunk(chunks_done)
                chunks_done += 1

    nc.compile()
    return nc


# ---------------------------------------------------------------- host side
def _pack_core(signals, sig_length, k1w, k1aw, k1ab, k2w, k3w,
               Wf, bf, Wb, bb, Wd, bd, core, TS):
    is_bw = core >= 4
    seqs0 = 8 * (core % 4)
    sig = signals[seqs0:seqs0 + 8, :, 0]
    L = sig_length[seqs0:seqs0 + 8].astype(np.int64)

    y = np.zeros((LANES, TS), np.float32)
    if not is_bw:
        y[:, :T] = sig
    else:
        for l in range(LANES):
            Ll = int(L[l])
            y[l, 0] = sig[l, Ll] if Ll < T else 0.0
            y[l, 1:1 + Ll] = sig[l, :Ll][::-1]
    SIGP = TS + 2
    ypad = np.zeros((LANES, SIGP), np.float32)
    ypad[:, 1:TS + 1] = y

    k1 = k1w[0, 0].astype(np.float32)      # [256]
    k1a = k1aw[0, 0].astype(np.float32)
    k1abv = k1ab.astype(np.float32)
    c1t = np.maximum(k1[:, None, None] * ypad[None], 0.0)
    c1t = c1t.reshape(2, 128, LANES * SIGP).astype(NP16)
    enc0 = np.maximum(k1a[:, None, None] * ypad[None, :, 1:TS + 1]
                      + k1abv[:, None, None], 0.0)
    enc0 = enc0.reshape(2, 128, LANES * TS).astype(NP16)

    W = (Wb if is_bw else Wf).astype(np.float32)
    bvec = (bb if is_bw else bf).astype(np.float32).copy()
    Wx = W[:C]
    Wh = W[C:]
    gsel = [1, 0, 2, 3]  # j, i, f, o
    Wx_p = np.zeros((C, G4), np.float32)
    Wh_p = np.zeros((H, G4), np.float32)
    gb = np.zeros((G4,), np.float32)
    for gi, g in enumerate(gsel):
        Wx_p[:, gi * GH:gi * GH + H] = Wx[:, g * H:(g + 1) * H]
        Wh_p[:, gi * GH:gi * GH + H] = Wh[:, g * H:(g + 1) * H]
        gb[gi * GH:gi * GH + H] = bvec[g * H:(g + 1) * H]
    gb[2 * GH:2 * GH + H] += 1.0           # forget_bias (f = gate idx 2)
    # j-gate pre-scaled x2: tanh(x) = 2*sigmoid(2x) - 1 on device
    Wx_p[:, 0:GH] *= 2.0
    Wh_p[:, 0:GH] *= 2.0
    gb[0:GH] *= 2.0

    k2 = (k2w[::-1] if is_bw else k2w).astype(np.float32)   # [3, 256, 256]
    k3 = k3w[0].astype(np.float32)                          # [256, 256]
    Wd_half = (Wd[H:] if is_bw else Wd[:H]).astype(np.float32)  # [200, 5]

    wh_packed = np.zeros((2, 128, G4), NP16)
    wh_packed[0] = Wh_p[0:128]
    wh_packed[1, 0:H - 128] = Wh_p[128:H]
    wd_packed = np.zeros((2, 128, 5), NP16)
    wd_packed[0] = Wd_half[0:128]
    wd_packed[1, 0:H - 128] = Wd_half[128:H]

    patch = np.zeros((128, 2, 8), NP16)
    if is_bw:
        patch[:] = -40.0

    return {
        "c1t": c1t,
        "enc0": enc0,
        "k2": np.ascontiguousarray(
            k2.reshape(3, 2, 128, 2, 128).transpose(3, 0, 1, 2, 4)).astype(NP16),
        "k3": np.ascontiguousarray(
            k3.reshape(2, 128, 2, 128).transpose(2, 0, 1, 3)).astype(NP16),
        "wx": np.ascontiguousarray(Wx_p.reshape(2, 128, G4)).astype(NP16),
        "wh": wh_packed,
        "gb": np.ascontiguousarray(gb.reshape(8, 128).T).astype(np.float32),
        "wd": wd_packed,
        "patch": patch,
        "id128": np.eye(128, dtype=NP16),
    }, L


def kernel(signals, sig_length, k1w, k1aw, k1ab, k2w, k3w, Wf, bf, Wb, bb, Wd, bd):
    from concourse.bass_utils import run_bass_kernel_spmd

    TS = T + 1
    CT = 128
    signals = np.asarray(signals, np.float32)
    sig_length = np.asarray(sig_length).astype(np.int64)
    args = [np.asarray(a, np.float32) for a in
            (k1w, k1aw, k1ab, k2w, k3w, Wf, bf, Wb, bb, Wd, bd)]
    k1w, k1aw, k1ab, k2w, k3w, Wf, bf, Wb, bb, Wd, bd = args

    key = (TS, CT)
    if key not in _CACHE:
        _CACHE[key] = _build(TS, CT)
    nc = _CACHE[key]

    in_maps = []
    Ls = []
    for core in range(8):
        m, L = _pack_core(signals, sig_length, k1w, k1aw, k1ab, k2w, k3w,
                          Wf, bf, Wb, bb, Wd, bd, core, TS)
        in_maps.append(m)
        Ls.append(L)

    res = run_bass_kernel_spmd(nc, in_maps, core_ids=list(range(8)))
    parts = [res.results[c]["part"] for c in range(8)]

    logits = np.zeros((B, T, 5), np.float32)
    t_idx = np.arange(T)
    for core in range(8):
        part = parts[core]
        seqs0 = 8 * (core % 4)
        for l in range(LANES):
            b = seqs0 + l
            if core < 4:
                logits[b] += part[:, :T, l].T
            else:
                L = int(Ls[core][l])
                u = L - t_idx
                valid = u >= 1
                logits[b, valid] += part[:, u[valid], l].T
    logits += bd[None, None, :].astype(np.float32)
    for b in range(B):
        L = int(sig_length[b])
        logits[b, L:] = bd
    return logits.astype(np.float32)
